# revision 1
# baseline (speedup 1.0000x reference)
"""Trainium2 Bass kernel for nn_NeuralTensorDiagLayer.

Computes out = tanh(concat([e1, e2], -1) @ V + diag + b) where
diag[k] = (sum_b(e1*e2) @ W[k]) / (B*D), broadcast over batch.

Sharding (8 NeuronCores, 2D: 4 batch groups x 2 k_out halves):
  - Core c handles batch rows [1024*(c//2), 1024*(c//2+1)) and k_out
    columns [1024*(c%2), 1024*(c%2+1)). Compared to pure batch-parallel,
    this halves the dominant V stream (16 MiB instead of 32 MiB per core;
    38 MiB total HBM traffic per core vs 46 MiB).
  - e1/e2 shards are fed pre-transposed to [feat, batch] by the host and
    held resident in SBUF (reused by both the matmul and the diag path);
    V arrives feature-major so no device transposes are needed.
  - diag: fused-on-DVE partial sum_b(e1*e2) per core, AllReduce over all
    8 cores (each batch row is counted twice -> 0.5 folded into the scale),
    then each core computes a 256-row diag slice against its W^T shard and
    an AllGather over the subgroups [[0,2,4,6],[1,3,5,7]] assembles each
    k_out half. The diag slice assignment is permuted (sc = (c%2)*4 + c//2,
    applied host-side when slicing W and b) so each subgroup gathers
    exactly its half in rank order - the device program stays SPMD-identical.
  - Main matmul runs in float32r (TensorE reduced-precision fp32 mode, 4x
    the fp32 throughput at ~12-bit mantissa accuracy); PSUM accumulation is
    fp32. V tiles are reused across both 512-wide batch chunks so each V
    element is read exactly once. PSUM is drained by DVE copies so the
    TensorEngine never waits on the diag collective chain; tanh+bias runs
    on the ScalarEngine afterwards.

Output is produced transposed ([k_out, batch] per core); the host
transposes/concats the 4x2 block grid back to (B, K).
"""

import os
import sys

for _p in ("/opt/trn_rl_repo", "/root/.axon_site/_ro/trn_rl_repo"):
    if os.path.isdir(_p) and _p not in sys.path:
        sys.path.append(_p)

import numpy as np

N_CORES = 8
B, D, K_OUT = 4096, 2048, 2048
FEAT = 2 * D
BG, KH = 4, 2                 # batch groups x kout halves
BPC = B // BG                 # 1024 batch rows per core
KHC = K_OUT // KH             # 1024 kout cols per core
KPC = K_OUT // N_CORES        # 256 diag rows per core
FT = FEAT // 128              # 32 feature tiles
DT = D // 128                 # 16 e1-space feature tiles
KTL = KHC // 128              # 8 local kout tiles
KGROUPS = (3, 3, 2)           # kout tile groups (2*g live PSUM banks)
DIAG_SCALE = 0.5 / (B * D)    # 0.5: the 8-core allreduce double-counts rows

_CACHE = {}


def _build_nc():
    import concourse.bacc as bacc
    import concourse.tile as tile
    import concourse.mybir as mybir

    repeat = int(os.environ.get("KERNEL_REPEAT", "1"))
    no_cc = bool(int(os.environ.get("KERNEL_NO_CC", "0")))
    dt = mybir.dt
    nc = bacc.Bacc("TRN2", target_bir_lowering=False, debug=False,
                   num_devices=N_CORES)

    e1t = nc.dram_tensor("e1t", [D, BPC], dt.float32r, kind="ExternalInput").ap()
    e2t = nc.dram_tensor("e2t", [D, BPC], dt.float32r, kind="ExternalInput").ap()
    v = nc.dram_tensor("v", [FEAT, KHC], dt.float32r, kind="ExternalInput").ap()
    wt = nc.dram_tensor("wt", [D, KPC], dt.float32, kind="ExternalInput").ap()
    bvec = nc.dram_tensor("bvec", [1, KPC], dt.float32, kind="ExternalInput").ap()
    out = nc.dram_tensor("out", [KHC, BPC], dt.float32, kind="ExternalOutput").ap()

    core_ids = list(range(N_CORES))
    ag_groups = [[0, 2, 4, 6], [1, 3, 5, 7]]

    with tile.TileContext(nc) as tc:
        with tc.tile_pool(name="xpool", bufs=1) as xpool, \
             tc.tile_pool(name="vpool", bufs=4) as vpool, \
             tc.tile_pool(name="wpool", bufs=4) as wpool, \
             tc.tile_pool(name="spool", bufs=1) as spool, \
             tc.tile_pool(name="scratch", bufs=2) as scratch, \
             tc.tile_pool(name="stage", bufs=1) as stage_pool, \
             tc.tile_pool(name="opool", bufs=2) as opool, \
             tc.tile_pool(name="psum", bufs=7, space="PSUM") as pp, \
             tc.tile_pool(name="psd", bufs=1, space="PSUM") as ppd, \
             tc.tile_pool(name="dram", bufs=1, space="DRAM") as dram:

            # ---- resident X^T = [e1^T ; e2^T] : 32 tiles of [128, BPC] ----
            x_all = xpool.tile([128, FT * BPC], dt.float32r)
            for j in range(DT):
                nc.sync.dma_start(x_all[:, j * BPC:(j + 1) * BPC],
                                  e1t[j * 128:(j + 1) * 128, :])
            for j in range(DT):
                jj = DT + j
                nc.sync.dma_start(x_all[:, jj * BPC:(jj + 1) * BPC],
                                  e2t[j * 128:(j + 1) * 128, :])

            # ---- partial s = sum_batch(e1*e2) on DVE ----
            # (tensor_tensor_reduce would fuse these but crashes the device)
            s_sb = spool.tile([128, DT], dt.float32)
            for j in range(DT):
                prod = scratch.tile([128, BPC], dt.float32, tag="prod",
                                    name=f"prod{j}")
                nc.vector.tensor_mul(
                    prod[:],
                    x_all[:, j * BPC:(j + 1) * BPC].bitcast(dt.float32),
                    x_all[:, (DT + j) * BPC:(DT + j + 1) * BPC].bitcast(dt.float32))
                nc.vector.tensor_reduce(s_sb[:, j:j + 1], prod[:],
                                        mybir.AxisListType.X,
                                        mybir.AluOpType.add)

            # ---- AllReduce s over all cores (8 KiB) ----
            s_in = dram.tile([128, DT], dt.float32)
            s_out = dram.tile([128, DT], dt.float32,
                              addr_space="Local" if no_cc else "Shared")
            nc.sync.dma_start(s_in[:], s_sb[:])
            if no_cc:
                nc.sync.dma_start(s_out[:], s_in[:])
            else:
                nc.gpsimd.collective_compute(
                    "AllReduce", mybir.AluOpType.add,
                    replica_groups=[core_ids],
                    ins=[s_in.opt()], outs=[s_out.opt()])
            s_r = spool.tile([128, DT], dt.float32, name="s_r")
            nc.sync.dma_start(s_r[:], s_out[:])

            # ---- diag slice: [1, KPC] = s @ wt (fp32 matmuls, M=1) ----
            b_sb = spool.tile([1, KPC], dt.float32, name="b_sb")
            nc.sync.dma_start(b_sb[:], bvec[:])
            diag_sb = spool.tile([1, KPC], dt.float32, name="diag_sb")
            ps_d = ppd.tile([1, KPC], dt.float32)
            for j in range(DT):
                wt_t = wpool.tile([128, KPC], dt.float32, tag="wt", name=f"wt{j}")
                nc.sync.dma_start(wt_t[:], wt[j * 128:(j + 1) * 128, :])
                nc.tensor.matmul(ps_d[:], s_r[:, j:j + 1], wt_t[:],
                                 start=(j == 0), stop=(j == DT - 1))
            nc.vector.tensor_scalar_mul(diag_sb[:], ps_d[:], DIAG_SCALE)
            nc.vector.tensor_add(diag_sb[:], diag_sb[:], b_sb[:])

            # ---- AllGather diag within the kout-half subgroup (1 KiB) ----
            d_in = dram.tile([1, KPC], dt.float32, name="d_in")
            d_out = dram.tile([KTL, 128], dt.float32, name="d_out")
            nc.sync.dma_start(d_in[:], diag_sb[:])
            if no_cc:
                for i in range(4):
                    nc.sync.dma_start(
                        d_out[2 * i:2 * i + 2, :],
                        d_in[:].rearrange("a (x p) -> (a x) p", p=128))
            else:
                nc.gpsimd.collective_compute(
                    "AllGather", mybir.AluOpType.bypass,
                    replica_groups=ag_groups,
                    ins=[d_in.opt()], outs=[d_out.opt()])
            # load as [128, KTL]: partition p, col k  <-  diag_half[k*128 + p]
            diag_cols = spool.tile([128, KTL], dt.float32, name="diag_cols")
            nc.sync.dma_start(diag_cols[:], d_out[:].rearrange("k p -> p k"))

            # ---- main matmul: out^T = V_half^T @ X^T, f32r on TensorE ----
            stage = stage_pool.tile([128, KTL * BPC], dt.float32, name="stage")
            for _rep in range(repeat):
              k0 = 0
              for kg, g in enumerate(KGROUPS):
                pss = [[pp.tile([128, 512], dt.float32, tag="ps",
                                name=f"ps{_rep}_{kg}_{q}_{b2}")
                        for b2 in range(2)] for q in range(g)]
                for j in range(FT):
                    vt = vpool.tile([128, 3 * 128], dt.float32r, tag="vt",
                                    name=f"vt{_rep}_{kg}_{j}")
                    nc.sync.dma_start(
                        vt[:, :g * 128],
                        v[j * 128:(j + 1) * 128, k0 * 128:(k0 + g) * 128])
                    for q in range(g):
                        for b2 in range(2):
                            nc.tensor.matmul(
                                pss[q][b2][:],
                                vt[:, q * 128:(q + 1) * 128],
                                x_all[:, j * BPC + b2 * 512:j * BPC + (b2 + 1) * 512],
                                start=(j == 0), stop=(j == FT - 1))
                for q in range(g):
                    kt = k0 + q
                    for b2 in range(2):
                        nc.vector.tensor_copy(
                            stage[:, kt * BPC + b2 * 512:kt * BPC + (b2 + 1) * 512],
                            pss[q][b2][:])
                    ot = opool.tile([128, BPC], dt.float32, tag="ot",
                                    name=f"ot{_rep}_{kt}")
                    nc.scalar.activation(ot[:], stage[:, kt * BPC:(kt + 1) * BPC],
                                         mybir.ActivationFunctionType.Tanh,
                                         bias=diag_cols[:, kt:kt + 1])
                    nc.sync.dma_start(out[kt * 128:(kt + 1) * 128, :], ot[:])
                k0 += g

    nc.compile()
    return nc


def _get_nc():
    if "nc" not in _CACHE:
        _CACHE["nc"] = _build_nc()
    return _CACHE["nc"]


def make_in_maps(e1, e2, W, V, b):
    in_maps = []
    for c in range(N_CORES):
        g, h = c // 2, c % 2
        sc = h * 4 + g            # permuted diag-slice index (see module doc)
        rows = slice(g * BPC, (g + 1) * BPC)
        krows = slice(sc * KPC, (sc + 1) * KPC)
        in_maps.append({
            "e1t": np.ascontiguousarray(e1[rows].T),
            "e2t": np.ascontiguousarray(e2[rows].T),
            "v": np.ascontiguousarray(V[:, h * KHC:(h + 1) * KHC]),
            "wt": np.ascontiguousarray(W[krows].T),
            "bvec": b[krows].reshape(1, KPC),
        })
    return in_maps


def kernel(e1, e2, W, V, b):
    from concourse.bass_utils import run_bass_kernel_spmd

    e1 = np.asarray(e1, dtype=np.float32)
    e2 = np.asarray(e2, dtype=np.float32)
    W = np.asarray(W, dtype=np.float32)
    V = np.asarray(V, dtype=np.float32)
    b = np.asarray(b, dtype=np.float32)

    nc = _get_nc()
    res = run_bass_kernel_spmd(nc, make_in_maps(e1, e2, W, V, b),
                               list(range(N_CORES)))
    out = np.empty((B, K_OUT), dtype=np.float32)
    for c in range(N_CORES):
        g, h = c // 2, c % 2
        out[g * BPC:(g + 1) * BPC, h * KHC:(h + 1) * KHC] = res.results[c]["out"].T
    return out



# revision 3
# speedup vs baseline: 1.2554x; 1.2554x over previous
"""Trainium2 Bass kernel for nn_NeuralTensorDiagLayer.

Computes out = tanh(concat([e1, e2], -1) @ V + diag + b) where
diag[k] = (sum_b(e1*e2) @ W[k]) / (B*D), broadcast over batch.

Sharding (8 NeuronCores, 2D: 4 batch groups x 2 k_out halves):
  - Core c handles batch rows [1024*(c//2), 1024*(c//2+1)) and k_out
    columns [1024*(c%2), 1024*(c%2+1)).
  - All big streams are bf16 (host casts): X^T resident 8 MiB, V 8 MiB,
    W^T 1 MiB, out 2 MiB -> 19 MiB HBM traffic vs 109 us of PE work
    (bf16 matmul, 1 col/cycle @2.4GHz) => PE-bound design.
  - e1/e2 shards arrive pre-transposed [feat, batch]; their tile loads are
    interleaved pairwise with the group-0 V stream so the TensorEngine
    starts within ~1 us instead of waiting for the full X load.
  - V arrives packed per k-tile group as [128, 256] contiguous blocks
    (single-ktile groups pack j-pairs side by side) so every V DMA is a
    64 KB transfer with 512 B runs.
  - diag: fused-on-DVE partial sum_b(e1*e2) per core (bf16), AllReduce
    over all 8 cores (8 KiB, 0.5 folded into the scale for the
    double-counted rows), 16 bf16 [1,256] matmuls against W^T in a
    dedicated PSUM bank, AllGather over subgroups [[0,2,4,6],[1,3,5,7]]
    assembles each k_out half (slice assignment permuted host-side, see
    make_in_maps).
  - Main loop: k-tile groups (2,1,2,1,2) -> (4,2,4,2,4) PSUM banks from a
    7-bank pool; current + draining group never exceed 7 banks so the PE
    never stalls on PSUM. DVE drains PSUM to a bf16 stage (unconditional,
    fast) so the PE is decoupled from the diag collective chain; ScalarE
    applies tanh with the diag+b column as per-partition bias; out is
    written bf16 and upcast on the host.

Output is produced transposed ([k_out, batch] per core); the host
transposes/concats the 4x2 block grid back to (B, K).
"""

import os
import sys

for _p in ("/opt/trn_rl_repo", "/root/.axon_site/_ro/trn_rl_repo"):
    if os.path.isdir(_p) and _p not in sys.path:
        sys.path.append(_p)

import numpy as np

N_CORES = 8
B, D, K_OUT = 4096, 2048, 2048
FEAT = 2 * D
BG, KH = 4, 2                 # batch groups x kout halves
BPC = B // BG                 # 1024 batch rows per core
KHC = K_OUT // KH             # 1024 kout cols per core
KPC = K_OUT // N_CORES        # 256 diag rows per core
FT = FEAT // 128              # 32 feature tiles
DT = D // 128                 # 16 e1-space feature tiles
KTL = KHC // 128              # 8 local kout tiles
KGROUPS = (2, 1, 2, 1, 2)     # kout tiles per group (2x = live PSUM banks)
DIAG_SCALE = 0.5 / (B * D)    # 0.5: the 8-core allreduce double-counts rows

_CACHE = {}


def _build_nc():
    import concourse.bacc as bacc
    import concourse.tile as tile
    import concourse.mybir as mybir

    repeat = int(os.environ.get("KERNEL_REPEAT", "1"))
    no_cc = bool(int(os.environ.get("KERNEL_NO_CC", "0")))
    skip_diag = bool(int(os.environ.get("KERNEL_SKIP_DIAG", "0")))
    dt = mybir.dt
    nc = bacc.Bacc("TRN2", target_bir_lowering=False, debug=False,
                   num_devices=N_CORES)

    e1t = nc.dram_tensor("e1t", [D, BPC], dt.bfloat16, kind="ExternalInput").ap()
    e2t = nc.dram_tensor("e2t", [D, BPC], dt.bfloat16, kind="ExternalInput").ap()
    # packed V: per group g, FT*gw/2 row-blocks of [128, 256]
    vps = [nc.dram_tensor(f"vp{g}", [FT * gw * 64, 256],
                          dt.bfloat16, kind="ExternalInput").ap()
           for g, gw in enumerate(KGROUPS)]
    wt = nc.dram_tensor("wt", [D, KPC], dt.bfloat16, kind="ExternalInput").ap()
    bvec = nc.dram_tensor("bvec", [1, KPC], dt.float32, kind="ExternalInput").ap()
    out = nc.dram_tensor("out", [KHC, BPC], dt.bfloat16, kind="ExternalOutput").ap()

    core_ids = list(range(N_CORES))
    ag_groups = [[0, 2, 4, 6], [1, 3, 5, 7]]

    with tile.TileContext(nc) as tc:
        with tc.tile_pool(name="xpool", bufs=1) as xpool, \
             tc.tile_pool(name="vpool", bufs=6) as vpool, \
             tc.tile_pool(name="wpool", bufs=4) as wpool, \
             tc.tile_pool(name="spool", bufs=1) as spool, \
             tc.tile_pool(name="scratch", bufs=2) as scratch, \
             tc.tile_pool(name="stage", bufs=1) as stage_pool, \
             tc.tile_pool(name="opool", bufs=2) as opool, \
             tc.tile_pool(name="psum", bufs=7, space="PSUM") as pp, \
             tc.tile_pool(name="psd", bufs=1, space="PSUM") as ppd, \
             tc.tile_pool(name="dram", bufs=1, space="DRAM") as dram:

            # ---- resident X^T = [e1^T ; e2^T] : 32 tiles of [128, BPC] ----
            # loads are pairwise (e1_j, e2_j) so the DVE partial for the diag
            # term can start immediately and the group-0 V stream interleaves.
            x_all = xpool.tile([128, FT * BPC], dt.bfloat16)
            s_sb = spool.tile([128, DT], dt.float32)
            for j in range(DT):
                nc.sync.dma_start(x_all[:, j * BPC:(j + 1) * BPC],
                                  e1t[j * 128:(j + 1) * 128, :])
                jj = DT + j
                nc.sync.dma_start(x_all[:, jj * BPC:(jj + 1) * BPC],
                                  e2t[j * 128:(j + 1) * 128, :])
                if not skip_diag:
                    prod = scratch.tile([128, BPC], dt.bfloat16, tag="prod",
                                        name=f"prod{j}")
                    nc.vector.tensor_mul(
                        prod[:],
                        x_all[:, j * BPC:(j + 1) * BPC],
                        x_all[:, jj * BPC:(jj + 1) * BPC])
                    nc.vector.tensor_reduce(s_sb[:, j:j + 1], prod[:],
                                            mybir.AxisListType.X,
                                            mybir.AluOpType.add)

            # ---- diag chain: AllReduce s -> [1,KPC] slice -> AllGather ----
            b_sb = spool.tile([1, KPC], dt.float32, name="b_sb")
            nc.sync.dma_start(b_sb[:], bvec[:])
            diag_cols = spool.tile([128, KTL], dt.float32, name="diag_cols")
            if skip_diag:
                zt = spool.tile([KTL, 128], dt.float32, name="zt")
                nc.vector.tensor_scalar_mul(zt[:], zt[:], 0.0)
                nc.vector.tensor_add(
                    zt[:], zt[:],
                    b_sb[:].rearrange("a (x p) -> (a x) p", p=128))
                zd = dram.tile([KTL, 128], dt.float32, name="zd")
                nc.sync.dma_start(zd[:], zt[:])
                nc.sync.dma_start(diag_cols[:], zd[:].rearrange("k p -> p k"))
            else:
                s_in = dram.tile([128, DT], dt.float32)
                s_out = dram.tile([128, DT], dt.float32,
                                  addr_space="Local" if no_cc else "Shared")
                nc.sync.dma_start(s_in[:], s_sb[:])
                if no_cc:
                    nc.sync.dma_start(s_out[:], s_in[:])
                else:
                    nc.gpsimd.collective_compute(
                        "AllReduce", mybir.AluOpType.add,
                        replica_groups=[core_ids],
                        ins=[s_in.opt()], outs=[s_out.opt()])
                s_r = spool.tile([128, DT], dt.float32, name="s_r")
                nc.sync.dma_start(s_r[:], s_out[:])
                s_b = spool.tile([128, DT], dt.bfloat16, name="s_b")
                nc.vector.tensor_copy(s_b[:], s_r[:])

                diag_sb = spool.tile([1, KPC], dt.float32, name="diag_sb")
                ps_d = ppd.tile([1, KPC], dt.float32)
                for j in range(DT):
                    wt_t = wpool.tile([128, KPC], dt.bfloat16, tag="wt",
                                      name=f"wt{j}")
                    nc.sync.dma_start(wt_t[:], wt[j * 128:(j + 1) * 128, :])
                    nc.tensor.matmul(ps_d[:], s_b[:, j:j + 1], wt_t[:],
                                     start=(j == 0), stop=(j == DT - 1))
                nc.vector.tensor_scalar_mul(diag_sb[:], ps_d[:], DIAG_SCALE)
                nc.vector.tensor_add(diag_sb[:], diag_sb[:], b_sb[:])

                d_in = dram.tile([1, KPC], dt.float32, name="d_in")
                d_out = dram.tile([KTL, 128], dt.float32, name="d_out")
                nc.sync.dma_start(d_in[:], diag_sb[:])
                if no_cc:
                    for i in range(4):
                        nc.sync.dma_start(
                            d_out[2 * i:2 * i + 2, :],
                            d_in[:].rearrange("a (x p) -> (a x) p", p=128))
                else:
                    nc.gpsimd.collective_compute(
                        "AllGather", mybir.AluOpType.bypass,
                        replica_groups=ag_groups,
                        ins=[d_in.opt()], outs=[d_out.opt()])
                # load as [128, KTL]: partition p, col k <- diag_half[k*128+p]
                nc.sync.dma_start(diag_cols[:], d_out[:].rearrange("k p -> p k"))

            # ---- main matmul: out^T = V_half^T @ X^T, bf16 on TensorE ----
            stage = stage_pool.tile([128, KTL * BPC], dt.bfloat16, name="stage")
            for _rep in range(repeat):
              k0 = 0
              for kg, gw in enumerate(KGROUPS):
                pss = [[pp.tile([128, 512], dt.float32, tag="ps",
                                name=f"ps{_rep}_{kg}_{q}_{b2}")
                        for b2 in range(2)] for q in range(gw)]
                vp = vps[kg]
                nsteps = FT if gw == 2 else FT // 2
                for j in range(nsteps):
                    vt = vpool.tile([128, 256], dt.bfloat16, tag="vt",
                                    name=f"vt{_rep}_{kg}_{j}")
                    nc.sync.dma_start(vt[:], vp[j * 128:(j + 1) * 128, :])
                    # gw==2: block j covers f-tile j, ktiles (k0, k0+1)
                    # gw==1: block j covers f-tiles (2j, 2j+1), ktile k0
                    for sub in range(2):
                        if gw == 2:
                            q, ft = sub, j
                        else:
                            q, ft = 0, 2 * j + sub
                        stat = vt[:, sub * 128:(sub + 1) * 128]
                        for b2 in range(2):
                            nc.tensor.matmul(
                                pss[q][b2][:],
                                stat,
                                x_all[:, ft * BPC + b2 * 512:
                                      ft * BPC + (b2 + 1) * 512],
                                start=(ft == 0 if gw == 2 else
                                       (j == 0 and sub == 0)),
                                stop=(ft == FT - 1 if gw == 2 else
                                      (j == nsteps - 1 and sub == 1)))
                for q in range(gw):
                    kt = k0 + q
                    for b2 in range(2):
                        nc.vector.tensor_copy(
                            stage[:, kt * BPC + b2 * 512:
                                  kt * BPC + (b2 + 1) * 512],
                            pss[q][b2][:])
                    ot = opool.tile([128, BPC], dt.bfloat16, tag="ot",
                                    name=f"ot{_rep}_{kt}")
                    nc.scalar.activation(ot[:], stage[:, kt * BPC:(kt + 1) * BPC],
                                         mybir.ActivationFunctionType.Tanh,
                                         bias=diag_cols[:, kt:kt + 1])
                    nc.sync.dma_start(out[kt * 128:(kt + 1) * 128, :], ot[:])
                k0 += gw

    nc.compile()
    return nc


def _get_nc():
    if "nc" not in _CACHE:
        _CACHE["nc"] = _build_nc()
    return _CACHE["nc"]


def _bf16(x):
    import ml_dtypes
    return np.ascontiguousarray(x).astype(ml_dtypes.bfloat16)


def _pack_v(v_half_bf):
    """v_half_bf: [FEAT, KHC] bf16 -> list of per-group packed arrays."""
    packed = []
    k0 = 0
    for gw in KGROUPS:
        cols = v_half_bf[:, k0 * 128:(k0 + gw) * 128]     # [FEAT, gw*128]
        if gw == 2:
            packed.append(np.ascontiguousarray(cols))      # rows 512B
        else:
            blk = cols.reshape(FT, 128, 128)
            pairs = np.concatenate([blk[0::2], blk[1::2]], axis=2)  # [16,128,256]
            packed.append(np.ascontiguousarray(
                pairs.reshape(FT // 2 * 128, 256)))
        k0 += gw
    return packed


def make_in_maps(e1, e2, W, V, b):
    in_maps = []
    Wb = _bf16(W)
    Vb = _bf16(V)
    for c in range(N_CORES):
        g, h = c // 2, c % 2
        sc = h * 4 + g            # permuted diag-slice index (see module doc)
        rows = slice(g * BPC, (g + 1) * BPC)
        krows = slice(sc * KPC, (sc + 1) * KPC)
        vpacks = _pack_v(np.ascontiguousarray(Vb[:, h * KHC:(h + 1) * KHC]))
        im = {
            "e1t": _bf16(e1[rows].T),
            "e2t": _bf16(e2[rows].T),
            "wt": np.ascontiguousarray(Wb[krows].T),
            "bvec": np.ascontiguousarray(b[krows].reshape(1, KPC),
                                         dtype=np.float32),
        }
        for gi, vp in enumerate(vpacks):
            im[f"vp{gi}"] = vp
        in_maps.append(im)
    return in_maps


def kernel(e1, e2, W, V, b):
    from concourse.bass_utils import run_bass_kernel_spmd

    e1 = np.asarray(e1, dtype=np.float32)
    e2 = np.asarray(e2, dtype=np.float32)
    W = np.asarray(W, dtype=np.float32)
    V = np.asarray(V, dtype=np.float32)
    b = np.asarray(b, dtype=np.float32)

    nc = _get_nc()
    res = run_bass_kernel_spmd(nc, make_in_maps(e1, e2, W, V, b),
                               list(range(N_CORES)))
    out = np.empty((B, K_OUT), dtype=np.float32)
    for c in range(N_CORES):
        g, h = c // 2, c % 2
        out[g * BPC:(g + 1) * BPC, h * KHC:(h + 1) * KHC] = \
            res.results[c]["out"].astype(np.float32).T
    return out


# revision 11
# speedup vs baseline: 1.3995x; 1.1148x over previous
"""Trainium2 Bass kernel for nn_NeuralTensorDiagLayer.

Computes out = tanh(concat([e1, e2], -1) @ V + diag + b) where
diag[k] = (sum_b(e1*e2) @ W[k]) / (B*D), broadcast over batch.

Sharding (8 NeuronCores, 2D: 4 batch groups x 2 k_out halves):
  - Core c handles batch rows [1024*(c//2), 1024*(c//2+1)) and k_out
    columns [1024*(c%2), 1024*(c%2+1)).
  - All big streams are bf16 (host casts): X^T resident 8 MiB, V 8 MiB,
    W^T 1 MiB, out 2 MiB -> 19 MiB HBM traffic vs 109 us of PE work
    (bf16 matmul, 1 col/cycle @2.4GHz) => PE-bound design.
  - e1/e2 shards arrive pre-transposed [feat, batch]; their tile loads are
    interleaved pairwise with the group-0 V stream so the TensorEngine
    starts within ~1 us instead of waiting for the full X load.
  - V arrives packed per k-tile group as [128, 256] contiguous blocks
    (single-ktile groups pack j-pairs side by side) so every V DMA is a
    64 KB transfer with 512 B runs.
  - diag: fused-on-DVE partial sum_b(e1*e2) per core (bf16), AllReduce
    over all 8 cores (8 KiB, 0.5 folded into the scale for the
    double-counted rows), 16 bf16 [1,256] matmuls against W^T in a
    dedicated PSUM bank, AllGather over subgroups [[0,2,4,6],[1,3,5,7]]
    assembles each k_out half (slice assignment permuted host-side, see
    make_in_maps).
  - Main loop: k-tile groups (2,1,2,1,2) -> (4,2,4,2,4) PSUM banks from a
    7-bank pool; current + draining group never exceed 7 banks so the PE
    never stalls on PSUM. DVE drains PSUM to a bf16 stage (unconditional,
    fast) so the PE is decoupled from the diag collective chain; ScalarE
    applies tanh with the diag+b column as per-partition bias; out is
    written bf16 and upcast on the host.

Output is produced transposed ([k_out, batch] per core); the host
transposes/concats the 4x2 block grid back to (B, K).
"""

import os
import sys

for _p in ("/opt/trn_rl_repo", "/root/.axon_site/_ro/trn_rl_repo"):
    if os.path.isdir(_p) and _p not in sys.path:
        sys.path.append(_p)

import numpy as np

N_CORES = 8
B, D, K_OUT = 4096, 2048, 2048
FEAT = 2 * D
BG, KH = 4, 2                 # batch groups x kout halves
BPC = B // BG                 # 1024 batch rows per core
KHC = K_OUT // KH             # 1024 kout cols per core
KPC = K_OUT // N_CORES        # 256 diag rows per core
FT = FEAT // 128              # 32 feature tiles
DT = D // 128                 # 16 e1-space feature tiles
KTL = KHC // 128              # 8 local kout tiles
KGROUPS = (2, 1, 2, 1, 2)     # kout tiles per group (2x = live PSUM banks)
DIAG_SCALE = 0.5 / (B * D)    # 0.5: the 8-core allreduce double-counts rows

_CACHE = {}


def _build_nc():
    import concourse.bacc as bacc
    import concourse.tile as tile
    import concourse.mybir as mybir

    repeat = int(os.environ.get("KERNEL_REPEAT", "1"))
    no_cc = bool(int(os.environ.get("KERNEL_NO_CC", "0")))
    skip_diag = bool(int(os.environ.get("KERNEL_SKIP_DIAG", "0")))
    with_tick = bool(int(os.environ.get("KERNEL_TICK", "0")))
    dt = mybir.dt
    nc = bacc.Bacc("TRN2", target_bir_lowering=False, debug=False,
                   num_devices=N_CORES)

    e1t = nc.dram_tensor("e1t", [D, BPC], dt.bfloat16, kind="ExternalInput").ap()
    e2t = nc.dram_tensor("e2t", [D, BPC], dt.bfloat16, kind="ExternalInput").ap()
    # packed V: per group g, FT*gw/2 row-blocks of [128, 256]
    vps = [nc.dram_tensor(f"vp{g}", [FT * gw * 64, 256],
                          dt.bfloat16, kind="ExternalInput").ap()
           for g, gw in enumerate(KGROUPS)]
    wt = nc.dram_tensor("wt", [D, KPC], dt.bfloat16, kind="ExternalInput").ap()
    bvec = nc.dram_tensor("bvec", [1, KPC], dt.float32, kind="ExternalInput").ap()
    out = nc.dram_tensor("out", [KHC, BPC], dt.bfloat16, kind="ExternalOutput").ap()
    tick = (nc.dram_tensor("tick", [1, KTL], dt.bfloat16,
                           kind="ExternalOutput").ap() if with_tick else None)

    core_ids = list(range(N_CORES))
    ag_groups = [[0, 2, 4, 6], [1, 3, 5, 7]]

    with tile.TileContext(nc) as tc:
        with tc.tile_pool(name="xpool", bufs=1) as xpool, \
             tc.tile_pool(name="vpool", bufs=6) as vpool, \
             tc.tile_pool(name="wpool", bufs=4) as wpool, \
             tc.tile_pool(name="spool", bufs=1) as spool, \
             tc.tile_pool(name="scratch", bufs=2) as scratch, \
             tc.tile_pool(name="stage", bufs=1) as stage_pool, \
             tc.tile_pool(name="opool", bufs=2) as opool, \
             tc.tile_pool(name="psum", bufs=7, space="PSUM") as pp, \
             tc.tile_pool(name="psd", bufs=1, space="PSUM") as ppd, \
             tc.tile_pool(name="dram", bufs=1, space="DRAM") as dram:

            # ---- resident X^T = [e1^T ; e2^T] : 32 tiles of [128, BPC] ----
            # x tile DMAs are interleaved into group 0's j-loop (each tile
            # lands just before its first matmul) so the TensorEngine starts
            # within ~1 us of kickoff instead of after the full 8 MiB load.
            # Separate tiles (not one big buffer): the Tile framework tracks
            # dependencies per tile, so a single x_all would stall the first
            # matmul on the LAST x DMA.
            x_tiles = [xpool.tile([128, BPC], dt.bfloat16, name=f"x{ft}")
                       for ft in range(FT)]
            s_sb = spool.tile([128, DT], dt.float32)
            b_sb = spool.tile([1, KPC], dt.float32, name="b_sb")
            nc.sync.dma_start(b_sb[:], bvec[:])
            diag_cols = spool.tile([128, KTL], dt.float32, name="diag_cols")

            def emit_x_step(ft):
                """DMA x tile ft; once both halves of a pair are resident,
                run the DVE partial for the diag term."""
                if ft < DT:
                    nc.sync.dma_start(x_tiles[ft][:],
                                      e1t[ft * 128:(ft + 1) * 128, :])
                else:
                    j = ft - DT
                    nc.sync.dma_start(x_tiles[ft][:],
                                      e2t[j * 128:(j + 1) * 128, :])
                    if not skip_diag:
                        prod = scratch.tile([128, BPC], dt.bfloat16,
                                            tag="prod", name=f"prod{j}")
                        nc.vector.tensor_mul(
                            prod[:], x_tiles[j][:], x_tiles[ft][:])
                        nc.vector.tensor_reduce(s_sb[:, j:j + 1], prod[:],
                                                mybir.AxisListType.X,
                                                mybir.AluOpType.add)

            def emit_diag_chain():
                # AllReduce s -> [1,KPC] diag slice -> AllGather -> diag_cols
                if skip_diag:
                    zt = spool.tile([KTL, 128], dt.float32, name="zt")
                    nc.vector.tensor_scalar_mul(zt[:], zt[:], 0.0)
                    nc.vector.tensor_add(
                        zt[:], zt[:],
                        b_sb[:].rearrange("a (x p) -> (a x) p", p=128))
                    zd = dram.tile([KTL, 128], dt.float32, name="zd")
                    nc.sync.dma_start(zd[:], zt[:])
                    nc.sync.dma_start(diag_cols[:],
                                      zd[:].rearrange("k p -> p k"))
                    return
                s_in = dram.tile([128, DT], dt.float32)
                s_out = dram.tile([128, DT], dt.float32,
                                  addr_space="Local" if no_cc else "Shared")
                nc.sync.dma_start(s_in[:], s_sb[:])
                if no_cc:
                    nc.sync.dma_start(s_out[:], s_in[:])
                else:
                    nc.gpsimd.collective_compute(
                        "AllReduce", mybir.AluOpType.add,
                        replica_groups=[core_ids],
                        ins=[s_in.opt()], outs=[s_out.opt()])
                s_r = spool.tile([128, DT], dt.float32, name="s_r")
                nc.sync.dma_start(s_r[:], s_out[:])
                s_b = spool.tile([128, DT], dt.bfloat16, name="s_b")
                nc.vector.tensor_copy(s_b[:], s_r[:])

                diag_sb = spool.tile([1, KPC], dt.float32, name="diag_sb")
                ps_d = ppd.tile([1, KPC], dt.float32)
                for j in range(DT):
                    wt_t = wpool.tile([128, KPC], dt.bfloat16, tag="wt",
                                      name=f"wt{j}")
                    nc.sync.dma_start(wt_t[:], wt[j * 128:(j + 1) * 128, :])
                    nc.tensor.matmul(ps_d[:], s_b[:, j:j + 1], wt_t[:],
                                     start=(j == 0), stop=(j == DT - 1))
                nc.vector.tensor_scalar_mul(diag_sb[:], ps_d[:], DIAG_SCALE)
                nc.vector.tensor_add(diag_sb[:], diag_sb[:], b_sb[:])

                d_in = dram.tile([1, KPC], dt.float32, name="d_in")
                d_out = dram.tile([KTL, 128], dt.float32, name="d_out")
                nc.sync.dma_start(d_in[:], diag_sb[:])
                if no_cc:
                    for i in range(4):
                        nc.sync.dma_start(
                            d_out[2 * i:2 * i + 2, :],
                            d_in[:].rearrange("a (x p) -> (a x) p", p=128))
                else:
                    nc.gpsimd.collective_compute(
                        "AllGather", mybir.AluOpType.bypass,
                        replica_groups=ag_groups,
                        ins=[d_in.opt()], outs=[d_out.opt()])
                # load as [128, KTL]: partition p, col k <- diag_half[k*128+p]
                nc.sync.dma_start(diag_cols[:],
                                  d_out[:].rearrange("k p -> p k"))

            # ---- main matmul: out^T = V_half^T @ X^T, bf16 on TensorE ----
            stage = stage_pool.tile([128, KTL * BPC], dt.bfloat16, name="stage")
            for _rep in range(repeat):
              k0 = 0
              for kg, gw in enumerate(KGROUPS):
                pss = [[pp.tile([128, 512], dt.float32, tag="ps",
                                name=f"ps{_rep}_{kg}_{q}_{b2}")
                        for b2 in range(2)] for q in range(gw)]
                vp = vps[kg]
                nsteps = FT if gw == 2 else FT // 2
                first_pass = (_rep == 0 and kg == 0)
                for j in range(nsteps):
                    if first_pass:
                        emit_x_step(j)
                    vt = vpool.tile([128, 256], dt.bfloat16, tag="vt",
                                    name=f"vt{_rep}_{kg}_{j}")
                    nc.sync.dma_start(vt[:], vp[j * 128:(j + 1) * 128, :])
                    # gw==2: block j covers f-tile j, ktiles (k0, k0+1)
                    # gw==1: block j covers f-tiles (2j, 2j+1), ktile k0
                    for sub in range(2):
                        if gw == 2:
                            q, ft = sub, j
                        else:
                            q, ft = 0, 2 * j + sub
                        stat = vt[:, sub * 128:(sub + 1) * 128]
                        for b2 in range(2):
                            nc.tensor.matmul(
                                pss[q][b2][:],
                                stat,
                                x_tiles[ft][:, b2 * 512:(b2 + 1) * 512],
                                start=(ft == 0 if gw == 2 else
                                       (j == 0 and sub == 0)),
                                stop=(ft == FT - 1 if gw == 2 else
                                      (j == nsteps - 1 and sub == 1)))
                if first_pass:
                    # emitted here so def-before-use holds for diag_cols'
                    # producers while the DAG still overlaps the collective
                    # chain with groups 1+; the drain below decouples the PE.
                    emit_diag_chain()
                for q in range(gw):
                    kt = k0 + q
                    for b2 in range(2):
                        nc.vector.tensor_copy(
                            stage[:, kt * BPC + b2 * 512:
                                  kt * BPC + (b2 + 1) * 512],
                            pss[q][b2][:])
                    ot = opool.tile([128, BPC], dt.bfloat16, tag="ot",
                                    name=f"ot{_rep}_{kt}")
                    nc.scalar.activation(ot[:], stage[:, kt * BPC:(kt + 1) * BPC],
                                         mybir.ActivationFunctionType.Tanh,
                                         bias=diag_cols[:, kt:kt + 1])
                    nc.sync.dma_start(out[kt * 128:(kt + 1) * 128, :], ot[:])
                    if with_tick and _rep == repeat - 1:
                        nc.sync.dma_start(tick[0:1, kt:kt + 1], ot[0:1, 0:1])
                k0 += gw

    nc.compile()
    return nc


def _get_nc():
    if "nc" not in _CACHE:
        _CACHE["nc"] = _build_nc()
    return _CACHE["nc"]


def _bf16(x):
    import ml_dtypes
    return np.ascontiguousarray(x).astype(ml_dtypes.bfloat16)


def _pack_v(v_half_bf):
    """v_half_bf: [FEAT, KHC] bf16 -> list of per-group packed arrays."""
    packed = []
    k0 = 0
    for gw in KGROUPS:
        cols = v_half_bf[:, k0 * 128:(k0 + gw) * 128]     # [FEAT, gw*128]
        if gw == 2:
            packed.append(np.ascontiguousarray(cols))      # rows 512B
        else:
            blk = cols.reshape(FT, 128, 128)
            pairs = np.concatenate([blk[0::2], blk[1::2]], axis=2)  # [16,128,256]
            packed.append(np.ascontiguousarray(
                pairs.reshape(FT // 2 * 128, 256)))
        k0 += gw
    return packed


def make_in_maps(e1, e2, W, V, b):
    in_maps = []
    Wb = _bf16(W)
    Vb = _bf16(V)
    for c in range(N_CORES):
        g, h = c // 2, c % 2
        sc = h * 4 + g            # permuted diag-slice index (see module doc)
        rows = slice(g * BPC, (g + 1) * BPC)
        krows = slice(sc * KPC, (sc + 1) * KPC)
        vpacks = _pack_v(np.ascontiguousarray(Vb[:, h * KHC:(h + 1) * KHC]))
        im = {
            "e1t": _bf16(e1[rows].T),
            "e2t": _bf16(e2[rows].T),
            "wt": np.ascontiguousarray(Wb[krows].T),
            "bvec": np.ascontiguousarray(b[krows].reshape(1, KPC),
                                         dtype=np.float32),
        }
        for gi, vp in enumerate(vpacks):
            im[f"vp{gi}"] = vp
        in_maps.append(im)
    return in_maps


def kernel(e1, e2, W, V, b):
    from concourse.bass_utils import run_bass_kernel_spmd

    e1 = np.asarray(e1, dtype=np.float32)
    e2 = np.asarray(e2, dtype=np.float32)
    W = np.asarray(W, dtype=np.float32)
    V = np.asarray(V, dtype=np.float32)
    b = np.asarray(b, dtype=np.float32)

    nc = _get_nc()
    res = run_bass_kernel_spmd(nc, make_in_maps(e1, e2, W, V, b),
                               list(range(N_CORES)))
    out = np.empty((B, K_OUT), dtype=np.float32)
    for c in range(N_CORES):
        g, h = c // 2, c % 2
        out[g * BPC:(g + 1) * BPC, h * KHC:(h + 1) * KHC] = \
            res.results[c]["out"].astype(np.float32).T
    return out


# revision 20
# speedup vs baseline: 1.9656x; 1.4044x over previous
"""Trainium2 Bass kernel for nn_NeuralTensorDiagLayer.

Computes out = tanh(concat([e1, e2], -1) @ V + diag + b) where
diag[k] = (sum_b(e1*e2) @ W[k]) / (B*D), broadcast over batch.

Sharding (8 NeuronCores, 2D: 4 batch groups x 2 k_out halves):
  - Core c handles batch rows [1024*(c//2), 1024*(c//2+1)) and k_out
    columns [1024*(c%2), 1024*(c%2+1)).
  - All big streams are bf16 (host casts): X^T resident 8 MiB, V 8 MiB,
    W^T 1 MiB, out 2 MiB -> 19 MiB HBM traffic vs 109 us of PE work
    (bf16 matmul, 1 col/cycle @2.4GHz) => PE-bound design.
  - DMA count is minimized (the HWDGE descriptor path costs ~0.6 us per
    DMA and was the hidden serializer): V arrives as 32 pre-packed
    [128, 1024] SBUF images (4 f-steps each), X as 16 [128, 2048]
    pair-images interleaved into group 0's stream so the TensorEngine
    starts within ~2 us, W^T as a single [128, 4096] image.
  - diag: fused-on-DVE partial sum_b(e1*e2) per core (bf16), AllReduce
    over all 8 cores (8 KiB, 0.5 folded into the scale for the
    double-counted rows), 16 bf16 [1,256] matmuls against W^T in a
    dedicated PSUM bank, AllGather over subgroups [[0,2,4,6],[1,3,5,7]]
    assembles each k_out half (slice assignment permuted host-side, see
    make_in_maps). The 16 PE matmuls sit between groups 1 and 2 in the
    in-order PE stream (not after group 0) so the PE never waits on the
    collective; drains of groups 0/1 are emitted after the chain so
    def-before-use holds for the diag bias.
  - Main loop: k-tile groups (2,1,2,1,2) -> (4,2,4,2,4) PSUM banks from a
    7-bank pool; current + draining group never exceed 7 banks so the PE
    never stalls on PSUM. DVE drains PSUM to a bf16 stage (unconditional,
    fast) so the PE is decoupled from the diag collective chain; ScalarE
    applies tanh with the diag+b column as per-partition bias; out is
    written bf16 and upcast on the host.

Output is produced transposed ([k_out, batch] per core); the host
transposes/concats the 4x2 block grid back to (B, K).
"""

import os
import sys

for _p in ("/opt/trn_rl_repo", "/root/.axon_site/_ro/trn_rl_repo"):
    if os.path.isdir(_p) and _p not in sys.path:
        sys.path.append(_p)

import numpy as np

N_CORES = 8
B, D, K_OUT = 4096, 2048, 2048
FEAT = 2 * D
BG, KH = 4, 2                 # batch groups x kout halves
BPC = B // BG                 # 1024 batch rows per core
KHC = K_OUT // KH             # 1024 kout cols per core
KPC = K_OUT // N_CORES        # 256 diag rows per core
FT = FEAT // 128              # 32 feature tiles
DT = D // 128                 # 16 e1-space feature tiles
KTL = KHC // 128              # 8 local kout tiles
KGROUPS = (2, 1, 2, 1, 1, 1)  # kout tiles per group (2x = live PSUM banks)
DIAG_SCALE = 0.5 / (B * D)    # 0.5: the 8-core allreduce double-counts rows

_CACHE = {}


def _build_nc():
    import concourse.bacc as bacc
    import concourse.tile as tile
    import concourse.mybir as mybir

    repeat = int(os.environ.get("KERNEL_REPEAT", "1"))
    no_cc = bool(int(os.environ.get("KERNEL_NO_CC", "0")))
    skip_diag = bool(int(os.environ.get("KERNEL_SKIP_DIAG", "0")))
    with_tick = bool(int(os.environ.get("KERNEL_TICK", "0")))
    dt = mybir.dt
    nc = bacc.Bacc("TRN2", target_bir_lowering=False, debug=False,
                   num_devices=N_CORES)

    # x pair-images: row-block jp is the SBUF image [128, 2*BPC] holding
    # f-tiles (2jp, 2jp+1); V megatile-images: row-block m of vp{g} is the
    # SBUF image [128, 1024] holding that group's f-steps 4m..4m+3;
    # W^T image: [128, DT*KPC].
    xp = nc.dram_tensor("xp", [DT * 128, 2 * BPC], dt.bfloat16,
                        kind="ExternalInput").ap()
    vps = [nc.dram_tensor(f"vp{g}", [FT * gw * 16, 1024], dt.bfloat16,
                          kind="ExternalInput").ap()
           for g, gw in enumerate(KGROUPS)]
    wtp = nc.dram_tensor("wtp", [128, DT * KPC], dt.bfloat16,
                         kind="ExternalInput").ap()
    bvec = nc.dram_tensor("bvec", [1, KPC], dt.float32, kind="ExternalInput").ap()
    out = nc.dram_tensor("out", [KHC, BPC], dt.bfloat16, kind="ExternalOutput").ap()
    tick = (nc.dram_tensor("tick", [1, KTL], dt.bfloat16,
                           kind="ExternalOutput").ap() if with_tick else None)

    core_ids = list(range(N_CORES))
    ag_groups = [[0, 2, 4, 6], [1, 3, 5, 7]]

    with tile.TileContext(nc) as tc:
        with tc.tile_pool(name="xpool", bufs=1) as xpool, \
             tc.tile_pool(name="vpool", bufs=4) as vpool, \
             tc.tile_pool(name="spool", bufs=1) as spool, \
             tc.tile_pool(name="scratch", bufs=2) as scratch, \
             tc.tile_pool(name="stage", bufs=1) as stage_pool, \
             tc.tile_pool(name="opool", bufs=2) as opool, \
             tc.tile_pool(name="psum", bufs=7, space="PSUM") as pp, \
             tc.tile_pool(name="psd", bufs=1, space="PSUM") as ppd, \
             tc.tile_pool(name="dram", bufs=1, space="DRAM") as dram:

            x_pairs = [xpool.tile([128, 2 * BPC], dt.bfloat16, name=f"xq{jp}")
                       for jp in range(DT)]

            def x_sl(ft, lo, hi):
                t = x_pairs[ft // 2]
                off = (ft % 2) * BPC
                return t[:, off + lo:off + hi]

            s_sb = spool.tile([128, DT], dt.float32)
            diag_cols = spool.tile([128, KTL], dt.float32, name="diag_cols")

            def emit_x_pair(jp):
                """DMA x pair-image jp; then any DVE diag partials whose two
                f-tiles are now resident (pairs 8.. complete e2-side)."""
                nc.sync.dma_start(x_pairs[jp][:],
                                  xp[jp * 128:(jp + 1) * 128, :])
                if not skip_diag and jp >= DT // 2:
                    for j in (2 * (jp - DT // 2), 2 * (jp - DT // 2) + 1):
                        prod = scratch.tile([128, BPC], dt.bfloat16,
                                            tag="prod", name=f"prod{j}")
                        nc.vector.tensor_mul(
                            prod[:], x_sl(j, 0, BPC), x_sl(DT + j, 0, BPC))
                        nc.vector.tensor_reduce(s_sb[:, j:j + 1], prod[:],
                                                mybir.AxisListType.X,
                                                mybir.AluOpType.add)

            def emit_diag_chain():
                # AllReduce s -> [1,KPC] diag slice -> AllGather -> diag_cols
                if skip_diag:
                    return
                b_sb = spool.tile([1, KPC], dt.float32, name="b_sb")
                nc.sync.dma_start(b_sb[:], bvec[:])
                s_in = dram.tile([128, DT], dt.float32)
                s_out = dram.tile([128, DT], dt.float32,
                                  addr_space="Local" if no_cc else "Shared")
                nc.sync.dma_start(s_in[:], s_sb[:])
                if no_cc:
                    nc.sync.dma_start(s_out[:], s_in[:])
                else:
                    nc.gpsimd.collective_compute(
                        "AllReduce", mybir.AluOpType.add,
                        replica_groups=[core_ids],
                        ins=[s_in.opt()], outs=[s_out.opt()])
                s_r = spool.tile([128, DT], dt.float32, name="s_r")
                nc.sync.dma_start(s_r[:], s_out[:])
                s_b = spool.tile([128, DT], dt.bfloat16, name="s_bf")
                nc.vector.tensor_copy(s_b[:], s_r[:])

                w_img = spool.tile([128, DT * KPC], dt.bfloat16, name="w_img")
                nc.sync.dma_start(w_img[:], wtp[:])
                diag_sb = spool.tile([1, KPC], dt.float32, name="diag_sb")
                ps_d = ppd.tile([1, KPC], dt.float32)
                for j in range(DT):
                    nc.tensor.matmul(ps_d[:], s_b[:, j:j + 1],
                                     w_img[:, j * KPC:(j + 1) * KPC],
                                     start=(j == 0), stop=(j == DT - 1))
                nc.vector.tensor_scalar_mul(diag_sb[:], ps_d[:], DIAG_SCALE)
                nc.vector.tensor_add(diag_sb[:], diag_sb[:], b_sb[:])

                d_in = dram.tile([1, KPC], dt.float32, name="d_in")
                d_out = dram.tile([KTL, 128], dt.float32, name="d_out")
                nc.sync.dma_start(d_in[:], diag_sb[:])
                if no_cc:
                    for i in range(4):
                        nc.sync.dma_start(
                            d_out[2 * i:2 * i + 2, :],
                            d_in[:].rearrange("a (x p) -> (a x) p", p=128))
                else:
                    nc.gpsimd.collective_compute(
                        "AllGather", mybir.AluOpType.bypass,
                        replica_groups=ag_groups,
                        ins=[d_in.opt()], outs=[d_out.opt()])
                # load as [128, KTL]: partition p, col k <- diag_half[k*128+p]
                nc.sync.dma_start(diag_cols[:],
                                  d_out[:].rearrange("k p -> p k"))

            # ---- main matmul: out^T = V_half^T @ X^T, bf16 on TensorE ----
            stage = stage_pool.tile([128, KTL * BPC], dt.bfloat16, name="stage")

            def emit_drain(_rep, k0, gw, pss):
                for q in range(gw):
                    kt = k0 + q
                    for b2 in range(2):
                        nc.vector.tensor_copy(
                            stage[:, kt * BPC + b2 * 512:
                                  kt * BPC + (b2 + 1) * 512],
                            pss[q][b2][:])
                    ot = opool.tile([128, BPC], dt.bfloat16, tag="ot",
                                    name=f"ot{_rep}_{kt}")
                    if skip_diag:
                        nc.scalar.activation(
                            ot[:], stage[:, kt * BPC:(kt + 1) * BPC],
                            mybir.ActivationFunctionType.Tanh)
                    else:
                        nc.scalar.activation(
                            ot[:], stage[:, kt * BPC:(kt + 1) * BPC],
                            mybir.ActivationFunctionType.Tanh,
                            bias=diag_cols[:, kt:kt + 1])
                    nc.sync.dma_start(out[kt * 128:(kt + 1) * 128, :], ot[:])
                    if with_tick and _rep == repeat - 1:
                        nc.sync.dma_start(tick[0:1, kt:kt + 1], ot[0:1, 0:1])

            for _rep in range(repeat):
              k0 = 0
              deferred = []
              for kg, gw in enumerate(KGROUPS):
                pss = [[pp.tile([128, 512], dt.float32, tag="ps",
                                name=f"ps{_rep}_{kg}_{q}_{b2}")
                        for b2 in range(2)] for q in range(gw)]
                vp = vps[kg]
                first_pass = (_rep == 0 and kg == 0)
                nmega = FT * gw // 8      # megatiles of 4 f-steps each
                for m in range(nmega):
                    vt = vpool.tile([128, 1024], dt.bfloat16, tag="vt",
                                    name=f"vt{_rep}_{kg}_{m}")
                    nc.sync.dma_start(vt[:], vp[m * 128:(m + 1) * 128, :])
                    if first_pass:
                        emit_x_pair(2 * m)
                        emit_x_pair(2 * m + 1)
                    per = 8 // gw         # f-tiles per megatile
                    for i in range(per):
                        ft = per * m + i
                        for q in range(gw):
                            stat = vt[:, (i * gw + q) * 128:
                                      (i * gw + q + 1) * 128]
                            for b2 in range(2):
                                nc.tensor.matmul(
                                    pss[q][b2][:], stat,
                                    x_sl(ft, b2 * 512, (b2 + 1) * 512),
                                    start=(ft == 0),
                                    stop=(ft == FT - 1))
                if _rep == 0 and kg == 1:
                    emit_diag_chain()
                    for args in deferred:
                        emit_drain(*args)
                    deferred = []
                    emit_drain(_rep, k0, gw, pss)
                elif _rep == 0 and kg == 0:
                    deferred.append((_rep, k0, gw, pss))
                else:
                    emit_drain(_rep, k0, gw, pss)
                k0 += gw

    nc.compile()
    return nc


def _get_nc():
    if "nc" not in _CACHE:
        _CACHE["nc"] = _build_nc()
    return _CACHE["nc"]


def _bf16(x):
    import ml_dtypes
    return np.ascontiguousarray(x).astype(ml_dtypes.bfloat16)


def _pack_x(e1s, e2s):
    """e1s/e2s: [BPC, D] f32 slices -> [DT*128, 2*BPC] bf16 pair-images."""
    xt = np.concatenate([_bf16(e1s.T), _bf16(e2s.T)], axis=0)  # [FEAT, BPC]
    tiles = xt.reshape(FT, 128, BPC)
    pairs = np.concatenate([tiles[0::2], tiles[1::2]], axis=2)  # [16,128,2BPC]
    return np.ascontiguousarray(pairs.reshape(DT * 128, 2 * BPC))


def _pack_v(v_half_bf):
    """v_half_bf: [FEAT, KHC] bf16 -> per-group megatile images.

    gw==2 group at cols c0:c0+256: megatile m image [128, 1024] holds
    f-tiles 4m..4m+3 side by side (each [128, 256]).
    gw==1 group at cols c0:c0+128: megatile m image holds f-tiles
    8m..8m+7 (each [128, 128])."""
    packed = []
    k0 = 0
    for gw in KGROUPS:
        cols = v_half_bf[:, k0 * 128:(k0 + gw) * 128]     # [FEAT, gw*128]
        blk = cols.reshape(FT, 128, gw * 128)
        per = 8 // gw                                      # f-tiles per mega
        nmega = FT // per
        img = np.concatenate([blk[i::per] for i in range(per)], axis=2)
        packed.append(np.ascontiguousarray(img.reshape(nmega * 128, 1024)))
        k0 += gw
    return packed


def make_in_maps(e1, e2, W, V, b):
    in_maps = []
    Wb = _bf16(W)
    Vb = _bf16(V)
    for c in range(N_CORES):
        g, h = c // 2, c % 2
        sc = h * 4 + g            # permuted diag-slice index (see module doc)
        rows = slice(g * BPC, (g + 1) * BPC)
        krows = slice(sc * KPC, (sc + 1) * KPC)
        # W^T image: [128, DT*KPC], block j = W^T[j*128:(j+1)*128, :]
        wt = np.ascontiguousarray(Wb[krows].T)             # [D, KPC]
        w_img = np.ascontiguousarray(
            wt.reshape(DT, 128, KPC).transpose(1, 0, 2).reshape(128, DT * KPC))
        vpacks = _pack_v(np.ascontiguousarray(Vb[:, h * KHC:(h + 1) * KHC]))
        im = {
            "xp": _pack_x(e1[rows], e2[rows]),
            "wtp": w_img,
            "bvec": np.ascontiguousarray(b[krows].reshape(1, KPC),
                                         dtype=np.float32),
        }
        for gi, vp in enumerate(vpacks):
            im[f"vp{gi}"] = vp
        in_maps.append(im)
    return in_maps


def kernel(e1, e2, W, V, b):
    from concourse.bass_utils import run_bass_kernel_spmd

    e1 = np.asarray(e1, dtype=np.float32)
    e2 = np.asarray(e2, dtype=np.float32)
    W = np.asarray(W, dtype=np.float32)
    V = np.asarray(V, dtype=np.float32)
    b = np.asarray(b, dtype=np.float32)

    nc = _get_nc()
    res = run_bass_kernel_spmd(nc, make_in_maps(e1, e2, W, V, b),
                               list(range(N_CORES)))
    out = np.empty((B, K_OUT), dtype=np.float32)
    for c in range(N_CORES):
        g, h = c // 2, c % 2
        out[g * BPC:(g + 1) * BPC, h * KHC:(h + 1) * KHC] = \
            res.results[c]["out"].astype(np.float32).T
    return out
